# revision 9
# baseline (speedup 1.0000x reference)
"""Binary tree-LSTM (BinaryTokenTreeModel) Trainium2 kernel, v2.

Problem: complete binary tree, depth 15 (N=32767 nodes), tree-LSTM with
state size 2H=512, gates 4*2H=2048, vocab 32.  Reference processes nodes
leaves-first; node i's input state is the concat of the first H=256 dims
of its two children's states.

Strategy (8 NeuronCores):
  * Data-parallel over 8 subtrees rooted at the 8 level-3 nodes (7..14).
    Each core runs a level-synchronous scan over global levels 13..6 of
    its subtree (2040 nodes per core).  Host: leaf level (32-entry type
    table, zero arithmetic) and the 63-node top (levels 5..0, exact fp32).
  * VOCAB=32 => x_proj folded into the level matmul as a one-hot
    contraction block (K = 256+256+32 = 544).  Level 13 contracts K=96
    one-hots against a reparameterized table (children are leaves).
  * sigma-everywhere cell: tanh(x) = 2*sigmoid(2x) - 1 with the 2x folded
    into the g-gate weight columns, so ONE activation instruction covers a
    chunk's whole gate tile (ACT fixed cost is 352 cycles/instr), plus one
    sigmoid(2*c_new).  The affine (2p - s) runs on DVE scalar_tensor_tensor.
  * Gate column layout [i f o g]crit | [i f o g]defer.  Small levels
    (10..6) compute only the critical half inline; the deferred halves of
    all 255 small-level nodes batch into 2 trailing chunks.
  * f16 for weights, states, cell intermediates and outputs (PSUM/fp32
    accumulation); halves DVE cost and all DMA traffic.

Self-contained: hardcodes all shapes; only needs numpy + the concourse
(bass) toolchain that ships with the environment.
"""

import sys

for _p in ("/opt/trn_rl_repo", "/root/.axon_site/_ro/trn_rl_repo"):
    if _p not in sys.path:
        sys.path.append(_p)

import numpy as np

import concourse.bacc as bacc
import concourse.mybir as mybir
import concourse.tile as tile
from concourse.bass_utils import run_bass_kernel_spmd

F32 = mybir.dt.float32
F16 = mybir.dt.float16
AF = mybir.ActivationFunctionType
ALU = mybir.AluOpType

N_CORES = 8
N = 32767
H = 256
H2 = 512
G = 2048  # 4 * H2
V = 32
LEAF0 = (1 << 14) - 1  # 16383: first leaf node id

# Gate column order: [i_c f_c o_c g_c | i_d f_d o_d g_d], 256 each.
# (orig torch order i f g o; crit = state dims 0:256, defer = 256:512)
PERM2 = np.concatenate([
    np.arange(0, 256), np.arange(512, 768),          # i_c f_c
    np.arange(1536, 1792), np.arange(1024, 1280),    # o_c g_c
    np.arange(256, 512), np.arange(768, 1024),       # i_d f_d
    np.arange(1792, 2048), np.arange(1280, 1536),    # o_d g_d
])

# device levels and their out-row offsets
DEV_PLAN = [(13, 1024, 0), (12, 512, 1024), (11, 256, 1536), (10, 128, 1792),
            (9, 64, 1920), (8, 32, 1984), (7, 16, 2016), (6, 8, 2032)]
OUT_ROWS = 2048  # 2040 h rows + 8 rows of level-6 c_crit (cols 0:256)
OHS_OFF = {12: 0, 11: 512, 10: 768, 9: 896, 8: 960, 7: 992, 6: 1008}
OHS_W = 1016
SMALL_OFF = {9: 0, 8: 64, 7: 96, 6: 112}  # offsets in the shared small stor
SMALL_W = 120

_BUILT = None  # cached (nc, input_names)
LAST_RESULTS = None  # BassKernelResults of the most recent run (for profiling)


def _sigmoid(x):
    return 1.0 / (1.0 + np.exp(-x))


class _Stor:
    def __init__(self, sA0, sA1, sB0, sB1, cin):
        self.sA0, self.sA1, self.sB0, self.sB1, self.cin = sA0, sA1, sB0, sB1, cin


def _build_program(nc):
    din = {}
    for name, shape in [
        ("wk0", [128, G]), ("wk1", [128, G]), ("wk2", [128, G]), ("wk3", [128, G]),
        ("woh", [32, G]), ("w13", [96, G]),
        ("oh3", [96, 1024]), ("ohs", [32, OHS_W]),
        ("eye", [128, 128]), ("cin13", [1024, 512]),
    ]:
        din[name] = nc.dram_tensor(name, shape, F16, kind="ExternalInput").ap()
    out_d = nc.dram_tensor("out", [OUT_ROWS, 512], F16, kind="ExternalOutput").ap()

    sbh = lambda n, sh: nc.alloc_sbuf_tensor(n, sh, F16).ap()
    wk = [sbh(f"wk{i}_s", [128, G]) for i in range(4)]
    woh_s = sbh("woh_s", [32, G])
    w13_s = sbh("w13_s", [96, G])
    oh3_s = sbh("oh3_s", [96, 1024])
    ohs_s = sbh("ohs_s", [32, OHS_W])
    eye_s = sbh("eye_s", [128, 128])
    cin13_s = sbh("cin13_s", [128, 8 * 512])

    # stationary storage (children states feeding each level)
    stor = {}
    for L, M in [(12, 512), (11, 256), (10, 128)]:
        mk = lambda nm: sbh(f"{nm}_{L}", [128, M])
        stor[L] = _Stor(mk("sA0"), mk("sA1"), mk("sB0"), mk("sB1"),
                        sbh(f"cin_{L}", [128, (M // 128) * 512]))
    sA0s = sbh("sA0_sm", [128, SMALL_W])
    sA1s = sbh("sA1_sm", [128, SMALL_W])
    sB0s = sbh("sB0_sm", [128, SMALL_W])
    sB1s = sbh("sB1_sm", [128, SMALL_W])
    cin_sm = sbh("cin_sm", [SMALL_W, 512])
    for L in (9, 8, 7, 6):
        o, w = SMALL_OFF[L], 1 << (L - 3)  # node count per core at L
        # per-level cin at base partition 0 (TensorTensor requires equal
        # base partitions); cin_sm is a shadow copy for the batched defer
        stor[L] = _Stor(sA0s[:, o:o + w], sA1s[:, o:o + w],
                        sB0s[:, o:o + w], sB1s[:, o:o + w],
                        sbh(f"cin_{L}", [w, 512]))

    with tile.TileContext(nc) as tc:
        import contextlib

        with contextlib.ExitStack() as ctx:
            g_pool = ctx.enter_context(
                tc.tile_pool(name="g", bufs=2, space="PSUM"))
            sig_pool = ctx.enter_context(tc.tile_pool(name="sig", bufs=2))
            cell_pool = ctx.enter_context(tc.tile_pool(name="cell", bufs=2))

            # input loads, L13's operands first, split across two queues
            nc.sync.dma_start(w13_s[0:48], din["w13"][0:48])
            nc.scalar.dma_start(w13_s[48:96], din["w13"][48:96])
            nc.sync.dma_start(oh3_s[0:48], din["oh3"][0:48])
            nc.scalar.dma_start(oh3_s[48:96], din["oh3"][48:96])
            nc.sync.dma_start(eye_s, din["eye"])
            for k in range(8):
                (nc.scalar if k % 2 else nc.sync).dma_start(
                    cin13_s[:, k * 512:(k + 1) * 512],
                    din["cin13"][k * 128:(k + 1) * 128, :])
            for d, s in [(din["wk0"], wk[0]), (din["wk2"], wk[2])]:
                nc.sync.dma_start(s, d)
            for d, s in [(din["wk1"], wk[1]), (din["wk3"], wk[3]),
                         (din["woh"], woh_s), (din["ohs"], ohs_s)]:
                nc.scalar.dma_start(s, d)

            # tiny junk matmuls start the HAM activity window early
            wtile = g_pool.tile([128, 2048], F32, tag="g")
            for _ in range(6):
                nc.tensor.matmul(wtile[0:128, 0:128], eye_s[:, 0:128],
                                 eye_s[:, 0:128], start=True, stop=True,
                                 skip_group_check=True)

            def emit(g, lhs, ws, wcol0, ncols, P):
                nk = len(lhs)
                nb = ncols // 512
                for k in range(nk):
                    for b in range(nb):
                        nc.tensor.matmul(
                            g[0:P, b * 512:(b + 1) * 512], lhs[k],
                            ws[k][:, wcol0 + b * 512:wcol0 + (b + 1) * 512],
                            start=(k == 0), stop=(k == nk - 1),
                            skip_group_check=True)

            def unit_fused(L, pk, row_off):
                P = 128
                c0 = pk * 128
                g = g_pool.tile([128, 2048], F32, tag="g")
                if L == 13:
                    lhs = [oh3_s[:, c0:c0 + P]]
                    ws = [w13_s]
                    cin_ap = cin13_s[0:P, pk * 512:(pk + 1) * 512]
                else:
                    st = stor[L]
                    lhs = [st.sA0[:, c0:c0 + P], st.sA1[:, c0:c0 + P],
                           st.sB0[:, c0:c0 + P], st.sB1[:, c0:c0 + P],
                           ohs_s[:, OHS_OFF[L] + c0:OHS_OFF[L] + c0 + P]]
                    ws = wk + [woh_s]
                    cin_ap = st.cin[0:P, pk * 512:(pk + 1) * 512]
                emit(g, lhs, ws, 0, 2048, P)

                sg = sig_pool.tile([128, 2048], F16, tag="sg")
                nc.scalar.activation(sg[0:P], g[0:P], AF.Sigmoid)
                s3 = sg[0:P].rearrange("p (j c) -> p j c", j=2)
                i3 = s3[:, :, 0:256]
                f3 = s3[:, :, 256:512]
                o3 = s3[:, :, 512:768]
                g3 = s3[:, :, 768:1024]
                cin3 = cin_ap.rearrange("p (j c) -> p j c", j=2)
                r3 = lambda t: t[0:P].rearrange("p (j c) -> p j c", j=2)

                p = cell_pool.tile([128, 512], F16, tag="p")
                nc.vector.tensor_mul(r3(p), i3, g3)
                pr = cell_pool.tile([128, 512], F16, tag="pr")
                nc.vector.scalar_tensor_tensor(r3(pr), r3(p), 2.0, i3,
                                               ALU.mult, ALU.subtract)
                q = cell_pool.tile([128, 512], F16, tag="q")
                nc.gpsimd.tensor_mul(r3(q), f3, cin3)
                cn = cell_pool.tile([128, 512], F16, tag="cn", bufs=3)
                nc.vector.tensor_add(cn[0:P], q[0:P], pr[0:P])
                sc = cell_pool.tile([128, 512], F16, tag="sc")
                nc.scalar.activation(sc[0:P], cn[0:P], AF.Sigmoid, scale=2.0)
                r = cell_pool.tile([128, 512], F16, tag="r")
                nc.gpsimd.tensor_mul(r3(r), o3, r3(sc))
                hn = cell_pool.tile([128, 512], F16, tag="hn", bufs=3)
                nc.vector.scalar_tensor_tensor(r3(hn), r3(r), 2.0, o3,
                                               ALU.mult, ALU.subtract)
                nc.sync.dma_start(out_d[row_off + c0:row_off + c0 + P, :],
                                  hn[0:P])
                return (g, hn, cn, P)

            def unit_half(lhs, cin_half, P, wcol0, out_ap):
                """Critical or deferred half: gates [i f o g] in cols 0:1024."""
                g = g_pool.tile([128, 2048], F32, tag="g")
                ws = wk + [woh_s]
                emit(g, lhs, ws, wcol0, 1024, P)
                sg = sig_pool.tile([128, 1024], F16, tag="sgh")
                nc.scalar.activation(sg[0:P], g[0:P, 0:1024], AF.Sigmoid)
                i_ = sg[0:P, 0:256]
                f_ = sg[0:P, 256:512]
                o_ = sg[0:P, 512:768]
                gg = sg[0:P, 768:1024]
                p = cell_pool.tile([128, 256], F16, tag="ph")
                nc.vector.tensor_mul(p[0:P], i_, gg)
                pr = cell_pool.tile([128, 256], F16, tag="prh")
                nc.vector.scalar_tensor_tensor(pr[0:P], p[0:P], 2.0, i_,
                                               ALU.mult, ALU.subtract)
                q = cell_pool.tile([128, 256], F16, tag="qh")
                nc.gpsimd.tensor_mul(q[0:P], f_, cin_half)
                cn = cell_pool.tile([128, 256], F16, tag="cnh", bufs=3)
                nc.vector.tensor_add(cn[0:P], q[0:P], pr[0:P])
                sc = cell_pool.tile([128, 256], F16, tag="sch")
                nc.scalar.activation(sc[0:P], cn[0:P], AF.Sigmoid, scale=2.0)
                r = cell_pool.tile([128, 256], F16, tag="rh")
                nc.gpsimd.tensor_mul(r[0:P], o_, sc[0:P])
                hn = cell_pool.tile([128, 256], F16, tag="hnh", bufs=3)
                nc.vector.scalar_tensor_tensor(hn[0:P], r[0:P], 2.0, o_,
                                               ALU.mult, ALU.subtract)
                nc.sync.dma_start(out_ap, hn[0:P])
                return (g, hn, cn, P)

            def feed(parent, u, ci, tcol, sm_off=None):
                """Write u's crit states into parent stationary storage.
                tcol: scratch column base in u's (dead) gates PSUM tile.
                sm_off: also shadow c into cin_sm at this row offset."""
                g, hn, cn, P = u
                half = P // 2
                base = ci * 64
                g16 = g.bitcast(F16)  # [128, 4096] f16 view of the PSUM tile
                tc2 = 2 * tcol
                t0 = g16[0:128, tc2:tc2 + P]
                nc.tensor.transpose(t0, hn[0:P, 0:128], eye_s[0:P, 0:P])
                t1 = g16[0:128, tc2 + 1024:tc2 + 1024 + P]
                nc.tensor.transpose(t1, hn[0:P, 128:256], eye_s[0:P, 0:P])
                nc.vector.tensor_copy(parent.sA0[:, base:base + half], t0[:, 0:P:2])
                nc.vector.tensor_copy(parent.sA1[:, base:base + half], t1[:, 0:P:2])
                nc.vector.tensor_copy(parent.sB0[:, base:base + half], t0[:, 1:P:2])
                nc.vector.tensor_copy(parent.sB1[:, base:base + half], t1[:, 1:P:2])
                dr = base % 128
                cb = (ci // 2) * 512
                nc.sync.dma_start(parent.cin[dr:dr + half, cb:cb + 256],
                                  cn[0:P:2, 0:256])
                nc.sync.dma_start(parent.cin[dr:dr + half, cb + 256:cb + 512],
                                  cn[1:P:2, 0:256])
                if sm_off is not None:
                    nc.scalar.dma_start(cin_sm[sm_off:sm_off + half, 0:256],
                                        cn[0:P:2, 0:256])
                    nc.scalar.dma_start(cin_sm[sm_off:sm_off + half, 256:512],
                                        cn[1:P:2, 0:256])

            def small_lhs(L):
                st = stor[L]
                M = 1 << (L - 3)
                return [st.sA0[:, 0:M], st.sA1[:, 0:M], st.sB0[:, 0:M],
                        st.sB1[:, 0:M],
                        ohs_s[:, OHS_OFF[L]:OHS_OFF[L] + M]]

            # ---- emission schedule (ring-safe: feed(u) within 1 unit) ----
            u13 = [None] * 8
            u13[0] = unit_fused(13, 0, 0)
            u13[1] = unit_fused(13, 1, 0)
            for c in range(2, 8):
                feed(stor[12], u13[c - 2], c - 2, 0)
                u13[c] = unit_fused(13, c, 0)
            feed(stor[12], u13[6], 6, 0)
            u12 = [None] * 4
            u12[0] = unit_fused(12, 0, 1024)
            feed(stor[12], u13[7], 7, 0)
            u12[1] = unit_fused(12, 1, 1024)
            feed(stor[11], u12[0], 0, 0)
            u12[2] = unit_fused(12, 2, 1024)
            feed(stor[11], u12[1], 1, 0)
            u12[3] = unit_fused(12, 3, 1024)
            feed(stor[11], u12[2], 2, 0)
            u11_0 = unit_fused(11, 0, 1536)
            feed(stor[11], u12[3], 3, 0)
            u11_1 = unit_fused(11, 1, 1536)
            feed(stor[10], u11_0, 0, 0)
            feed(stor[10], u11_1, 1, 0)

            # small levels: crit halves, feeding down; defers batched after
            u10 = unit_half(small_lhs(10), stor[10].cin[0:128, 0:256], 128,
                            0, out_d[1792:1920, 0:256])
            feed(stor[9], u10, 0, 1024, sm_off=SMALL_OFF[9])
            u9 = unit_half(small_lhs(9), stor[9].cin[0:64, 0:256], 64,
                           0, out_d[1920:1984, 0:256])
            feed(stor[8], u9, 0, 1024, sm_off=SMALL_OFF[8])
            u8 = unit_half(small_lhs(8), stor[8].cin[0:32, 0:256], 32,
                           0, out_d[1984:2016, 0:256])
            feed(stor[7], u8, 0, 1024, sm_off=SMALL_OFF[7])
            u7 = unit_half(small_lhs(7), stor[7].cin[0:16, 0:256], 16,
                           0, out_d[2016:2032, 0:256])
            feed(stor[6], u7, 0, 1024, sm_off=SMALL_OFF[6])
            u6 = unit_half(small_lhs(6), stor[6].cin[0:8, 0:256], 8,
                           0, out_d[2032:2040, 0:256])
            # level-6 c_crit rows for the host's top-of-tree chain
            nc.sync.dma_start(out_d[2040:2048, 0:256], u6[2][0:8, 0:256])

            # deferred halves: L10 as one chunk, L9..L6 as one 120-row chunk
            unit_half(small_lhs(10), stor[10].cin[0:128, 256:512], 128,
                      1024, out_d[1792:1920, 256:512])
            lhs2 = [sA0s[:, 0:SMALL_W], sA1s[:, 0:SMALL_W],
                    sB0s[:, 0:SMALL_W], sB1s[:, 0:SMALL_W],
                    ohs_s[:, 896:896 + SMALL_W]]
            unit_half(lhs2, cin_sm[0:SMALL_W, 256:512], SMALL_W,
                      1024, out_d[1920:2040, 256:512])

    nc.compile()
    return [k for k in din]


def _get_built():
    global _BUILT
    if _BUILT is None:
        nc = bacc.Bacc("TRN2", target_bir_lowering=False, debug=False,
                       num_devices=N_CORES)
        names = _build_program(nc)
        _BUILT = (nc, names)
    return _BUILT


def kernel(types, a_idx, b_idx, emb, W_ih, W_hh, b_ih, b_hh):
    types = np.asarray(types, np.int32)
    emb = np.asarray(emb, np.float32)
    W_ih = np.asarray(W_ih, np.float32)
    W_hh = np.asarray(W_hh, np.float32)
    b = np.asarray(b_ih, np.float32) + np.asarray(b_hh, np.float32)

    # ---- host weight reparameterization (O(V), no O(N) arithmetic) ----
    XT = (W_ih @ emb.T + b[:, None]).astype(np.float32)          # [2048, 32]
    c_leaf = _sigmoid(XT[0:512]) * np.tanh(XT[1024:1536])        # [512, 32]
    h_leaf = _sigmoid(XT[1536:2048]) * np.tanh(c_leaf)           # [512, 32]
    M_A = W_hh[:, 0:256] @ h_leaf[0:256]                         # [2048, 32]
    M_B = W_hh[:, 256:512] @ h_leaf[0:256]

    def dev_layout(mat_t):
        """[K, 2048] original gate cols -> PERM2 layout, g cols doubled, f16."""
        m = np.ascontiguousarray(mat_t[:, PERM2], np.float32)
        m[:, 768:1024] *= 2.0
        m[:, 1792:2048] *= 2.0
        return m.astype(np.float16)

    w13 = dev_layout(np.vstack([M_A.T, M_B.T, XT.T]))            # [96, 2048]
    W_augT = dev_layout(np.vstack([W_hh.T, XT.T]))               # [544, 2048]
    wkh = [np.ascontiguousarray(W_augT[i * 128:(i + 1) * 128])
           for i in range(4)]
    woh = np.ascontiguousarray(W_augT[512:544])
    cl256 = np.ascontiguousarray(c_leaf[0:256].T, np.float16)    # [32, 256]
    eye = np.eye(128, dtype=np.float16)

    in_maps = []
    for j in range(N_CORES):
        base13 = (1 << 13) - 1 + j * 1024
        n = np.arange(base13, base13 + 1024)
        oh3 = np.zeros((96, 1024), np.float16)
        m = np.arange(1024)
        oh3[types[2 * n + 1], m] = 1.0
        oh3[32 + types[2 * n + 2], m] = 1.0
        oh3[64 + types[n], m] = 1.0
        cin13 = np.concatenate(
            [cl256[types[2 * n + 1]], cl256[types[2 * n + 2]]],
            axis=1).astype(np.float16)
        ohs = np.zeros((32, OHS_W), np.float16)
        for L in range(12, 5, -1):
            mm = 1 << (L - 3)
            basel = (1 << L) - 1 + j * mm
            off = OHS_OFF[L]
            ohs[types[basel:basel + mm], off + np.arange(mm)] = 1.0
        in_maps.append({
            "wk0": wkh[0], "wk1": wkh[1], "wk2": wkh[2], "wk3": wkh[3],
            "woh": woh, "w13": w13, "cin13": cin13,
            "oh3": oh3, "ohs": ohs, "eye": eye,
        })

    nc, _ = _get_built()
    res = run_bass_kernel_spmd(nc, in_maps, core_ids=list(range(N_CORES)))
    global LAST_RESULTS
    LAST_RESULTS = res

    out = np.empty((N, H2), np.float32)
    for j in range(N_CORES):
        r = res.results[j]["out"].astype(np.float32)
        for (L, mm, off) in DEV_PLAN:
            basel = (1 << L) - 1 + j * mm
            out[basel:basel + mm] = r[off:off + mm]
    out[LEAF0:] = h_leaf.T[types[LEAF0:]]

    # top of tree (levels 5..0, 63 nodes) on host, mirroring the reference
    Hs = np.zeros((127, H2), np.float32)
    Cc = np.zeros((127, H), np.float32)  # c_crit only
    for j in range(N_CORES):
        r = res.results[j]["out"].astype(np.float32)
        Hs[63 + 8 * j:63 + 8 * j + 8] = r[2032:2040]
        Cc[63 + 8 * j:63 + 8 * j + 8] = r[2040:2048, 0:256]
    for L in range(5, -1, -1):
        n = np.arange((1 << L) - 1, (1 << (L + 1)) - 1)
        a, bb = 2 * n + 1, 2 * n + 2
        h_in = np.concatenate([Hs[a, :H], Hs[bb, :H]], axis=1)
        c_in = np.concatenate([Cc[a], Cc[bb]], axis=1)
        gates = XT[:, types[n]].T + h_in @ W_hh.T
        ig, fg, gg, og = np.split(gates, 4, axis=1)
        c_new = _sigmoid(fg) * c_in + _sigmoid(ig) * np.tanh(gg)
        h_new = _sigmoid(og) * np.tanh(c_new)
        Hs[n] = h_new
        Cc[n] = c_new[:, 0:256]
        out[n] = h_new
    return out


# revision 10
# speedup vs baseline: 1.2351x; 1.2351x over previous
"""Binary tree-LSTM (BinaryTokenTreeModel) Trainium2 kernel, v3.

Problem: complete binary tree, depth 15 (N=32767 nodes), tree-LSTM with
state size 2H=512, gates 4*2H=2048, vocab 32.  Reference processes nodes
leaves-first; node i's input state is the concat of the first H=256 dims
of its two children's states.

Strategy (8 NeuronCores):
  * Data-parallel over 8 subtrees rooted at the 8 level-3 nodes (7..14).
    Each core runs a level-synchronous scan over global levels 13..6 of
    its subtree (2040 nodes per core).  Host: leaf level (32-entry type
    table, zero arithmetic) and the 63-node top (levels 5..0, exact fp32).
  * VOCAB=32 => x_proj folded into the level matmul as a one-hot
    contraction block (K = 256+256+32 = 544).  Level 13 contracts K=96
    one-hots against a reparameterized table (children are leaves).
  * sigma-everywhere cell: tanh(x) = 2*sigmoid(2x) - 1 with the 2x folded
    into the g-gate weight columns, minimizing ACT instructions (352-cycle
    fixed cost each).  The affine (2p - s) runs on DVE scalar_tensor_tensor.
  * Gate column layout [i | f | o | g] (each 512 = crit 256 | defer 256) so
    every cell op is a flat contiguous f16 slice (DVE 2x 16-bit mode).
  * Small levels (10..6) compute only the critical half inline; deferred
    halves of all 255 small-level nodes batch into 2 trailing chunks.
  * PSUM: gates pool 3 x [128,1024]x2banks, transpose scratch pool
    2 x 1 bank -- feeds never block the matmul ring (the v1/v2 serializer).
  * Feed transposes use permuted identities so A/B-child columns come out
    blocked; all feed copies are contiguous.

Self-contained: hardcodes all shapes; only needs numpy + the concourse
(bass) toolchain that ships with the environment.
"""

import sys

for _p in ("/opt/trn_rl_repo", "/root/.axon_site/_ro/trn_rl_repo"):
    if _p not in sys.path:
        sys.path.append(_p)

import numpy as np

import concourse.bacc as bacc
import concourse.mybir as mybir
import concourse.tile as tile
from concourse.bass_utils import run_bass_kernel_spmd

F32 = mybir.dt.float32
F16 = mybir.dt.float16
AF = mybir.ActivationFunctionType
ALU = mybir.AluOpType

N_CORES = 8
N = 32767
H = 256
H2 = 512
G = 2048  # 4 * H2
V = 32
LEAF0 = (1 << 14) - 1  # 16383: first leaf node id

# Gate column order [i | f | o | g]; orig torch row order is i f g o.
PERM3 = np.concatenate([
    np.arange(0, 512), np.arange(512, 1024),
    np.arange(1536, 2048), np.arange(1024, 1536),
])

DEV_PLAN = [(13, 1024, 0), (12, 512, 1024), (11, 256, 1536), (10, 128, 1792),
            (9, 64, 1920), (8, 32, 1984), (7, 16, 2016), (6, 8, 2032)]
OUT_ROWS = 2048  # 2040 h rows + 8 rows of level-6 c_crit (cols 0:256)
OHS_OFF = {12: 0, 11: 512, 10: 768, 9: 896, 8: 960, 7: 992, 6: 1008}
OHS_W = 1016
SMALL_OFF = {9: 0, 8: 64, 7: 96, 6: 112}  # offsets in the shared small stor
SMALL_W = 120
EYP_OFF = {128: 0, 64: 128, 32: 192, 16: 224, 8: 240}
EYP_W = 248

_BUILT = None  # cached (nc, input_names)
LAST_RESULTS = None  # BassKernelResults of the most recent run (for profiling)


def _sigmoid(x):
    return 1.0 / (1.0 + np.exp(-x))


class _Stor:
    def __init__(self, sA0, sA1, sB0, sB1, cin):
        self.sA0, self.sA1, self.sB0, self.sB1, self.cin = sA0, sA1, sB0, sB1, cin


def _build_program(nc):
    din = {}
    for name, shape in [
        ("wk0", [128, G]), ("wk1", [128, G]), ("wk2", [128, G]), ("wk3", [128, G]),
        ("woh", [32, G]), ("w13", [96, G]),
        ("oh3", [96, 1024]), ("ohs", [32, OHS_W]),
        ("eyp", [128, EYP_W]), ("cin13", [1024, 512]),
    ]:
        din[name] = nc.dram_tensor(name, shape, F16, kind="ExternalInput").ap()
    out_d = nc.dram_tensor("out", [OUT_ROWS, 512], F16, kind="ExternalOutput").ap()

    sbh = lambda n, sh: nc.alloc_sbuf_tensor(n, sh, F16).ap()
    wk = [sbh(f"wk{i}_s", [128, G]) for i in range(4)]
    woh_s = sbh("woh_s", [32, G])
    w13_s = sbh("w13_s", [96, G])
    oh3_s = sbh("oh3_s", [96, 1024])
    ohs_s = sbh("ohs_s", [32, OHS_W])
    eyp_s = sbh("eyp_s", [128, EYP_W])
    cin13_s = sbh("cin13_s", [128, 8 * 512])

    stor = {}
    for L, M in [(12, 512), (11, 256), (10, 128)]:
        mk = lambda nm: sbh(f"{nm}_{L}", [128, M])
        stor[L] = _Stor(mk("sA0"), mk("sA1"), mk("sB0"), mk("sB1"),
                        sbh(f"cin_{L}", [128, (M // 128) * 512]))
    sA0s = sbh("sA0_sm", [128, SMALL_W])
    sA1s = sbh("sA1_sm", [128, SMALL_W])
    sB0s = sbh("sB0_sm", [128, SMALL_W])
    sB1s = sbh("sB1_sm", [128, SMALL_W])
    cin_sm = sbh("cin_sm", [SMALL_W, 512])
    for L in (9, 8, 7, 6):
        o, w = SMALL_OFF[L], 1 << (L - 3)
        stor[L] = _Stor(sA0s[:, o:o + w], sA1s[:, o:o + w],
                        sB0s[:, o:o + w], sB1s[:, o:o + w],
                        sbh(f"cin_{L}", [w, 512]))

    with tile.TileContext(nc) as tc:
        import contextlib

        with contextlib.ExitStack() as ctx:
            g_pool = ctx.enter_context(
                tc.tile_pool(name="g", bufs=3, space="PSUM"))
            tr_pool = ctx.enter_context(
                tc.tile_pool(name="tr", bufs=2, space="PSUM"))
            sig_pool = ctx.enter_context(tc.tile_pool(name="sig", bufs=2))
            cell_pool = ctx.enter_context(tc.tile_pool(name="cell", bufs=2))

            # input loads, L13's operands first, split across two queues
            nc.sync.dma_start(eyp_s, din["eyp"])
            nc.sync.dma_start(w13_s[0:48], din["w13"][0:48])
            nc.scalar.dma_start(w13_s[48:96], din["w13"][48:96])
            nc.sync.dma_start(oh3_s[0:48], din["oh3"][0:48])
            nc.scalar.dma_start(oh3_s[48:96], din["oh3"][48:96])
            for k in range(8):
                (nc.scalar if k % 2 else nc.sync).dma_start(
                    cin13_s[:, k * 512:(k + 1) * 512],
                    din["cin13"][k * 128:(k + 1) * 128, :])
            for d, s in [(din["wk0"], wk[0]), (din["wk2"], wk[2])]:
                nc.sync.dma_start(s, d)
            for d, s in [(din["wk1"], wk[1]), (din["wk3"], wk[3]),
                         (din["woh"], woh_s), (din["ohs"], ohs_s)]:
                nc.scalar.dma_start(s, d)

            # tiny junk matmuls start the HAM activity window early
            wtile = g_pool.tile([128, 1024], F32, tag="g")
            for _ in range(6):
                nc.tensor.matmul(wtile[0:128, 0:128], eyp_s[:, 0:128],
                                 eyp_s[:, 0:128], start=True, stop=True,
                                 skip_group_check=True)

            def emit_fused(gA, gB, lhs, ws, P):
                nk = len(lhs)
                for k in range(nk):
                    st, sp = k == 0, k == nk - 1
                    for gt, wc in ((gA, 0), (gA, 512), (gB, 1024), (gB, 1536)):
                        oc = wc % 1024
                        nc.tensor.matmul(gt[0:P, oc:oc + 512], lhs[k],
                                         ws[k][:, wc:wc + 512],
                                         start=st, stop=sp,
                                         skip_group_check=True)

            def emit_half(g, lhs, ws, dsel, P):
                nk = len(lhs)
                for k in range(nk):
                    st, sp = k == 0, k == nk - 1
                    for j, wc in enumerate((0, 512, 1024, 1536)):
                        w0 = wc + dsel * 256
                        nc.tensor.matmul(g[0:P, j * 256:(j + 1) * 256], lhs[k],
                                         ws[k][:, w0:w0 + 256],
                                         start=st, stop=sp,
                                         skip_group_check=True)

            def unit_fused(L, pk, row_off):
                P = 128
                c0 = pk * 128
                gA = g_pool.tile([128, 1024], F32, tag="g")
                gB = g_pool.tile([128, 1024], F32, tag="g")
                if L == 13:
                    lhs = [oh3_s[:, c0:c0 + P]]
                    ws = [w13_s]
                    cin_ap = cin13_s[0:P, pk * 512:(pk + 1) * 512]
                else:
                    st = stor[L]
                    lhs = [st.sA0[:, c0:c0 + P], st.sA1[:, c0:c0 + P],
                           st.sB0[:, c0:c0 + P], st.sB1[:, c0:c0 + P],
                           ohs_s[:, OHS_OFF[L] + c0:OHS_OFF[L] + c0 + P]]
                    ws = wk + [woh_s]
                    cin_ap = st.cin[0:P, pk * 512:(pk + 1) * 512]
                emit_fused(gA, gB, lhs, ws, P)

                sg = sig_pool.tile([128, 2048], F16, tag="sg")
                nc.scalar.activation(sg[0:P, 0:1024], gA[0:P], AF.Sigmoid)
                nc.scalar.activation(sg[0:P, 1024:2048], gB[0:P], AF.Sigmoid)
                i_ = sg[0:P, 0:512]
                f_ = sg[0:P, 512:1024]
                o_ = sg[0:P, 1024:1536]
                g_ = sg[0:P, 1536:2048]
                q = cell_pool.tile([128, 512], F16, tag="q")
                nc.gpsimd.tensor_mul(q[0:P], f_, cin_ap)
                p = cell_pool.tile([128, 512], F16, tag="p")
                nc.vector.tensor_mul(p[0:P], i_, g_)
                pr = cell_pool.tile([128, 512], F16, tag="pr")
                nc.vector.scalar_tensor_tensor(pr[0:P], p[0:P], 2.0, i_,
                                               ALU.mult, ALU.subtract)
                cn = cell_pool.tile([128, 512], F16, tag="cn", bufs=3)
                nc.vector.tensor_add(cn[0:P], q[0:P], pr[0:P])
                sc = cell_pool.tile([128, 512], F16, tag="sc")
                nc.scalar.activation(sc[0:P], cn[0:P], AF.Sigmoid, scale=2.0)
                r = cell_pool.tile([128, 512], F16, tag="r")
                nc.gpsimd.tensor_mul(r[0:P], o_, sc[0:P])
                hn = cell_pool.tile([128, 512], F16, tag="hn", bufs=3)
                nc.vector.scalar_tensor_tensor(hn[0:P], r[0:P], 2.0, o_,
                                               ALU.mult, ALU.subtract)
                nc.sync.dma_start(out_d[row_off + c0:row_off + c0 + P, :],
                                  hn[0:P])
                return (hn, cn, P)

            def unit_half(lhs, cin_half, P, dsel, out_ap):
                """Critical (dsel=0) or deferred (dsel=1) half of a small
                level; gates [i f o g] (256 each) in one 1024-col tile."""
                g = g_pool.tile([128, 1024], F32, tag="g")
                emit_half(g, lhs, wk + [woh_s], dsel, P)
                sg = sig_pool.tile([128, 1024], F16, tag="sgh")
                nc.scalar.activation(sg[0:P], g[0:P], AF.Sigmoid)
                i_ = sg[0:P, 0:256]
                f_ = sg[0:P, 256:512]
                o_ = sg[0:P, 512:768]
                gg = sg[0:P, 768:1024]
                p = cell_pool.tile([128, 256], F16, tag="ph")
                nc.vector.tensor_mul(p[0:P], i_, gg)
                pr = cell_pool.tile([128, 256], F16, tag="prh")
                nc.vector.scalar_tensor_tensor(pr[0:P], p[0:P], 2.0, i_,
                                               ALU.mult, ALU.subtract)
                q = cell_pool.tile([128, 256], F16, tag="qh")
                nc.vector.tensor_mul(q[0:P], f_, cin_half)
                cn = cell_pool.tile([128, 256], F16, tag="cnh", bufs=3)
                nc.vector.tensor_add(cn[0:P], q[0:P], pr[0:P])
                sc = cell_pool.tile([128, 256], F16, tag="sch")
                nc.scalar.activation(sc[0:P], cn[0:P], AF.Sigmoid, scale=2.0)
                r = cell_pool.tile([128, 256], F16, tag="rh")
                nc.vector.tensor_mul(r[0:P], o_, sc[0:P])
                hn = cell_pool.tile([128, 256], F16, tag="hnh", bufs=3)
                nc.vector.scalar_tensor_tensor(hn[0:P], r[0:P], 2.0, o_,
                                               ALU.mult, ALU.subtract)
                nc.sync.dma_start(out_ap, hn[0:P])
                return (hn, cn, P)

            def feed(parent, u, ci, sm_off=None):
                """Write u's crit states into parent stationary storage."""
                hn, cn, P = u
                half = P // 2
                base = ci * 64
                off = EYP_OFF[P]
                pid = eyp_s[0:P, off:off + P]
                t0 = tr_pool.tile([128, 256], F16, tag="t")
                nc.tensor.transpose(t0[0:128, 0:P], hn[0:P, 0:128], pid)
                t1 = tr_pool.tile([128, 256], F16, tag="t")
                nc.tensor.transpose(t1[0:128, 0:P], hn[0:P, 128:256], pid)
                nc.vector.tensor_copy(parent.sA0[:, base:base + half],
                                      t0[:, 0:half])
                nc.vector.tensor_copy(parent.sB0[:, base:base + half],
                                      t0[:, half:P])
                nc.vector.tensor_copy(parent.sA1[:, base:base + half],
                                      t1[:, 0:half])
                nc.vector.tensor_copy(parent.sB1[:, base:base + half],
                                      t1[:, half:P])
                dr = base % 128
                cb = (ci // 2) * 512
                nc.sync.dma_start(parent.cin[dr:dr + half, cb:cb + 256],
                                  cn[0:P:2, 0:256])
                nc.sync.dma_start(parent.cin[dr:dr + half, cb + 256:cb + 512],
                                  cn[1:P:2, 0:256])
                if sm_off is not None:
                    nc.scalar.dma_start(cin_sm[sm_off:sm_off + half, 0:256],
                                        cn[0:P:2, 0:256])
                    nc.scalar.dma_start(cin_sm[sm_off:sm_off + half, 256:512],
                                        cn[1:P:2, 0:256])

            def small_lhs(L):
                st = stor[L]
                M = 1 << (L - 3)
                return [st.sA0[:, 0:M], st.sA1[:, 0:M], st.sB0[:, 0:M],
                        st.sB1[:, 0:M],
                        ohs_s[:, OHS_OFF[L]:OHS_OFF[L] + M]]

            # ---- emission schedule (feeds lag one unit) ----
            u13 = [None] * 8
            u13[0] = unit_fused(13, 0, 0)
            u13[1] = unit_fused(13, 1, 0)
            for c in range(2, 8):
                feed(stor[12], u13[c - 2], c - 2)
                u13[c] = unit_fused(13, c, 0)
            feed(stor[12], u13[6], 6)
            u12 = [None] * 4
            u12[0] = unit_fused(12, 0, 1024)
            feed(stor[12], u13[7], 7)
            u12[1] = unit_fused(12, 1, 1024)
            feed(stor[11], u12[0], 0)
            u12[2] = unit_fused(12, 2, 1024)
            feed(stor[11], u12[1], 1)
            u12[3] = unit_fused(12, 3, 1024)
            feed(stor[11], u12[2], 2)
            u11_0 = unit_fused(11, 0, 1536)
            feed(stor[11], u12[3], 3)
            u11_1 = unit_fused(11, 1, 1536)
            feed(stor[10], u11_0, 0)
            feed(stor[10], u11_1, 1)

            # small levels: crit halves, feeding down; defers batched after
            u10 = unit_half(small_lhs(10), stor[10].cin[0:128, 0:256], 128,
                            0, out_d[1792:1920, 0:256])
            feed(stor[9], u10, 0, sm_off=SMALL_OFF[9])
            u9 = unit_half(small_lhs(9), stor[9].cin[0:64, 0:256], 64,
                           0, out_d[1920:1984, 0:256])
            feed(stor[8], u9, 0, sm_off=SMALL_OFF[8])
            u8 = unit_half(small_lhs(8), stor[8].cin[0:32, 0:256], 32,
                           0, out_d[1984:2016, 0:256])
            feed(stor[7], u8, 0, sm_off=SMALL_OFF[7])
            u7 = unit_half(small_lhs(7), stor[7].cin[0:16, 0:256], 16,
                           0, out_d[2016:2032, 0:256])
            feed(stor[6], u7, 0, sm_off=SMALL_OFF[6])
            u6 = unit_half(small_lhs(6), stor[6].cin[0:8, 0:256], 8,
                           0, out_d[2032:2040, 0:256])
            # level-6 c_crit rows for the host's top-of-tree chain
            nc.sync.dma_start(out_d[2040:2048, 0:256], u6[1][0:8, 0:256])

            # deferred halves: L10 as one chunk, L9..L6 as one 120-row chunk
            unit_half(small_lhs(10), stor[10].cin[0:128, 256:512], 128,
                      1, out_d[1792:1920, 256:512])
            lhs2 = [sA0s[:, 0:SMALL_W], sA1s[:, 0:SMALL_W],
                    sB0s[:, 0:SMALL_W], sB1s[:, 0:SMALL_W],
                    ohs_s[:, 896:896 + SMALL_W]]
            unit_half(lhs2, cin_sm[0:SMALL_W, 256:512], SMALL_W,
                      1, out_d[1920:2040, 256:512])

    nc.compile()
    return [k for k in din]


def _get_built():
    global _BUILT
    if _BUILT is None:
        nc = bacc.Bacc("TRN2", target_bir_lowering=False, debug=False,
                       num_devices=N_CORES)
        names = _build_program(nc)
        _BUILT = (nc, names)
    return _BUILT


def _make_eyp():
    eyp = np.zeros((128, EYP_W), np.float16)
    for P, off in EYP_OFF.items():
        half = P // 2
        for bcol in range(P):
            a = 2 * bcol if bcol < half else 2 * (bcol - half) + 1
            eyp[a, off + bcol] = 1.0
    return eyp


def kernel(types, a_idx, b_idx, emb, W_ih, W_hh, b_ih, b_hh):
    types = np.asarray(types, np.int32)
    emb = np.asarray(emb, np.float32)
    W_ih = np.asarray(W_ih, np.float32)
    W_hh = np.asarray(W_hh, np.float32)
    b = np.asarray(b_ih, np.float32) + np.asarray(b_hh, np.float32)

    # ---- host weight reparameterization (O(V), no O(N) arithmetic) ----
    XT = (W_ih @ emb.T + b[:, None]).astype(np.float32)          # [2048, 32]
    c_leaf = _sigmoid(XT[0:512]) * np.tanh(XT[1024:1536])        # [512, 32]
    h_leaf = _sigmoid(XT[1536:2048]) * np.tanh(c_leaf)           # [512, 32]
    M_A = W_hh[:, 0:256] @ h_leaf[0:256]                         # [2048, 32]
    M_B = W_hh[:, 256:512] @ h_leaf[0:256]

    def dev_layout(mat_t):
        """[K, 2048] original gate cols -> [i|f|o|g], g cols doubled, f16."""
        m = np.ascontiguousarray(mat_t[:, PERM3], np.float32)
        m[:, 1536:2048] *= 2.0
        return m.astype(np.float16)

    w13 = dev_layout(np.vstack([M_A.T, M_B.T, XT.T]))            # [96, 2048]
    W_augT = dev_layout(np.vstack([W_hh.T, XT.T]))               # [544, 2048]
    wkh = [np.ascontiguousarray(W_augT[i * 128:(i + 1) * 128])
           for i in range(4)]
    woh = np.ascontiguousarray(W_augT[512:544])
    cl256 = np.ascontiguousarray(c_leaf[0:256].T, np.float16)    # [32, 256]
    eyp = _make_eyp()

    in_maps = []
    for j in range(N_CORES):
        base13 = (1 << 13) - 1 + j * 1024
        n = np.arange(base13, base13 + 1024)
        oh3 = np.zeros((96, 1024), np.float16)
        m = np.arange(1024)
        oh3[types[2 * n + 1], m] = 1.0
        oh3[32 + types[2 * n + 2], m] = 1.0
        oh3[64 + types[n], m] = 1.0
        cin13 = np.concatenate(
            [cl256[types[2 * n + 1]], cl256[types[2 * n + 2]]],
            axis=1).astype(np.float16)
        ohs = np.zeros((32, OHS_W), np.float16)
        for L in range(12, 5, -1):
            mm = 1 << (L - 3)
            basel = (1 << L) - 1 + j * mm
            off = OHS_OFF[L]
            ohs[types[basel:basel + mm], off + np.arange(mm)] = 1.0
        in_maps.append({
            "wk0": wkh[0], "wk1": wkh[1], "wk2": wkh[2], "wk3": wkh[3],
            "woh": woh, "w13": w13, "cin13": cin13,
            "oh3": oh3, "ohs": ohs, "eyp": eyp,
        })

    nc, _ = _get_built()
    res = run_bass_kernel_spmd(nc, in_maps, core_ids=list(range(N_CORES)))
    global LAST_RESULTS
    LAST_RESULTS = res

    out = np.empty((N, H2), np.float32)
    for j in range(N_CORES):
        r = res.results[j]["out"].astype(np.float32)
        for (L, mm, off) in DEV_PLAN:
            basel = (1 << L) - 1 + j * mm
            out[basel:basel + mm] = r[off:off + mm]
    out[LEAF0:] = h_leaf.T[types[LEAF0:]]

    # top of tree (levels 5..0, 63 nodes) on host, mirroring the reference
    Hs = np.zeros((127, H2), np.float32)
    Cc = np.zeros((127, H), np.float32)  # c_crit only
    for j in range(N_CORES):
        r = res.results[j]["out"].astype(np.float32)
        Hs[63 + 8 * j:63 + 8 * j + 8] = r[2032:2040]
        Cc[63 + 8 * j:63 + 8 * j + 8] = r[2040:2048, 0:256]
    for L in range(5, -1, -1):
        n = np.arange((1 << L) - 1, (1 << (L + 1)) - 1)
        a, bb = 2 * n + 1, 2 * n + 2
        h_in = np.concatenate([Hs[a, :H], Hs[bb, :H]], axis=1)
        c_in = np.concatenate([Cc[a], Cc[bb]], axis=1)
        gates = XT[:, types[n]].T + h_in @ W_hh.T
        ig, fg, gg, og = np.split(gates, 4, axis=1)
        c_new = _sigmoid(fg) * c_in + _sigmoid(ig) * np.tanh(gg)
        h_new = _sigmoid(og) * np.tanh(c_new)
        Hs[n] = h_new
        Cc[n] = c_new[:, 0:256]
        out[n] = h_new
    return out


# revision 15
# speedup vs baseline: 1.4634x; 1.1848x over previous
"""Binary tree-LSTM (BinaryTokenTreeModel) Trainium2 kernel, v3.

Problem: complete binary tree, depth 15 (N=32767 nodes), tree-LSTM with
state size 2H=512, gates 4*2H=2048, vocab 32.  Reference processes nodes
leaves-first; node i's input state is the concat of the first H=256 dims
of its two children's states.

Strategy (8 NeuronCores):
  * Data-parallel over 8 subtrees rooted at the 8 level-3 nodes (7..14).
    Each core runs a level-synchronous scan over global levels 13..6 of
    its subtree (2040 nodes per core).  Host: leaf level (32-entry type
    table, zero arithmetic) and the 63-node top (levels 5..0, exact fp32).
  * VOCAB=32 => x_proj folded into the level matmul as a one-hot
    contraction block (K = 256+256+32 = 544).  Level 13 contracts K=96
    one-hots against a reparameterized table (children are leaves).
  * sigma-everywhere cell: tanh(x) = 2*sigmoid(2x) - 1 with the 2x folded
    into the g-gate weight columns, minimizing ACT instructions (352-cycle
    fixed cost each).  The affine (2p - s) runs on DVE scalar_tensor_tensor.
  * Gate column layout [i | f | o | g] (each 512 = crit 256 | defer 256) so
    every cell op is a flat contiguous f16 slice (DVE 2x 16-bit mode).
  * Small levels (10..6) compute only the critical half inline; deferred
    halves of all 255 small-level nodes batch into 2 trailing chunks.
  * PSUM: gates pool 3 x [128,1024]x2banks, transpose scratch pool
    2 x 1 bank -- feeds never block the matmul ring (the v1/v2 serializer).
  * Feed transposes use permuted identities so A/B-child columns come out
    blocked; all feed copies are contiguous.

Self-contained: hardcodes all shapes; only needs numpy + the concourse
(bass) toolchain that ships with the environment.
"""

import sys

for _p in ("/opt/trn_rl_repo", "/root/.axon_site/_ro/trn_rl_repo"):
    if _p not in sys.path:
        sys.path.append(_p)

import numpy as np

import concourse.bacc as bacc
import concourse.mybir as mybir
import concourse.tile as tile
from concourse.bass_utils import run_bass_kernel_spmd

F32 = mybir.dt.float32
F16 = mybir.dt.float16
AF = mybir.ActivationFunctionType
ALU = mybir.AluOpType

N_CORES = 8
N = 32767
H = 256
H2 = 512
G = 2048  # 4 * H2
V = 32
LEAF0 = (1 << 14) - 1  # 16383: first leaf node id

# Gate column order [i | f | o | g]; orig torch row order is i f g o.
PERM3 = np.concatenate([
    np.arange(0, 512), np.arange(512, 1024),
    np.arange(1536, 2048), np.arange(1024, 1536),
])

DEV_PLAN = [(13, 1024, 0), (12, 512, 1024), (11, 256, 1536), (10, 128, 1792),
            (9, 64, 1920), (8, 32, 1984), (7, 16, 2016), (6, 8, 2032)]
OUT_ROWS = 2048  # 2040 h rows + 8 rows of level-6 c_crit (cols 0:256)
OHS_OFF = {12: 0, 11: 512, 10: 768, 9: 896, 8: 960, 7: 992, 6: 1008}
OHS_W = 1016
SMALL_OFF = {9: 0, 8: 64, 7: 96, 6: 112}  # offsets in the shared small stor
SMALL_W = 120
EYP_OFF = {128: 0, 64: 128, 32: 192, 16: 224, 8: 240}
EYP_W = 248

_BUILT = None  # cached (nc, input_names)
LAST_RESULTS = None  # BassKernelResults of the most recent run (for profiling)


def _sigmoid(x):
    return 1.0 / (1.0 + np.exp(-x))


class _Stor:
    def __init__(self, sA0, sA1, sB0, sB1, cin):
        self.sA0, self.sA1, self.sB0, self.sB1, self.cin = sA0, sA1, sB0, sB1, cin


def _build_program(nc):
    din = {}
    for name, shape in [
        ("wk0", [128, G]), ("wk1", [128, G]), ("wk2", [128, G]), ("wk3", [128, G]),
        ("woh", [32, G]), ("w13", [96, G]),
        ("oh3", [96, 1024]), ("ohs", [32, OHS_W]),
        ("eyp", [128, EYP_W]), ("cin13", [1024, 512]),
    ]:
        din[name] = nc.dram_tensor(name, shape, F16, kind="ExternalInput").ap()
    out_d = nc.dram_tensor("out", [OUT_ROWS, 512], F16, kind="ExternalOutput").ap()

    sbh = lambda n, sh: nc.alloc_sbuf_tensor(n, sh, F16).ap()
    wk = [sbh(f"wk{i}_s", [128, G]) for i in range(4)]
    woh_s = sbh("woh_s", [32, G])
    w13_s = sbh("w13_s", [96, G])
    oh3_s = sbh("oh3_s", [96, 1024])
    ohs_s = sbh("ohs_s", [32, OHS_W])
    eyp_s = sbh("eyp_s", [128, EYP_W])
    cin13_s = sbh("cin13_s", [128, 8 * 512])

    stor = {}
    for L, M in [(12, 512), (11, 256), (10, 128)]:
        mk = lambda nm: sbh(f"{nm}_{L}", [128, M])
        stor[L] = _Stor(mk("sA0"), mk("sA1"), mk("sB0"), mk("sB1"),
                        sbh(f"cin_{L}", [128, (M // 128) * 512]))
    sA0s = sbh("sA0_sm", [128, SMALL_W])
    sA1s = sbh("sA1_sm", [128, SMALL_W])
    sB0s = sbh("sB0_sm", [128, SMALL_W])
    sB1s = sbh("sB1_sm", [128, SMALL_W])
    cin_sm = sbh("cin_sm", [SMALL_W, 512])
    for L in (9, 8, 7, 6):
        o, w = SMALL_OFF[L], 1 << (L - 3)
        stor[L] = _Stor(sA0s[:, o:o + w], sA1s[:, o:o + w],
                        sB0s[:, o:o + w], sB1s[:, o:o + w],
                        sbh(f"cin_{L}", [w, 512]))

    with tile.TileContext(nc) as tc:
        import contextlib

        with contextlib.ExitStack() as ctx:
            g_pool = ctx.enter_context(
                tc.tile_pool(name="g", bufs=3, space="PSUM"))
            tr_pool = ctx.enter_context(
                tc.tile_pool(name="tr", bufs=2, space="PSUM"))
            sig_pool = ctx.enter_context(tc.tile_pool(name="sig", bufs=4))
            cell_pool = ctx.enter_context(tc.tile_pool(name="cell", bufs=3))

            # input loads, L13's operands first, split across two queues
            nc.sync.dma_start(eyp_s, din["eyp"])
            nc.sync.dma_start(w13_s[0:48], din["w13"][0:48])
            nc.scalar.dma_start(w13_s[48:96], din["w13"][48:96])
            nc.sync.dma_start(oh3_s[0:48], din["oh3"][0:48])
            nc.scalar.dma_start(oh3_s[48:96], din["oh3"][48:96])
            for k in range(8):
                (nc.scalar if k % 2 else nc.sync).dma_start(
                    cin13_s[:, k * 512:(k + 1) * 512],
                    din["cin13"][k * 128:(k + 1) * 128, :])
            for d, s in [(din["wk0"], wk[0]), (din["wk2"], wk[2])]:
                nc.sync.dma_start(s, d)
            for d, s in [(din["wk1"], wk[1]), (din["wk3"], wk[3]),
                         (din["woh"], woh_s), (din["ohs"], ohs_s)]:
                nc.scalar.dma_start(s, d)

            # tiny junk matmuls start the HAM activity window early
            wtile = g_pool.tile([128, 1024], F32, tag="g")
            for _ in range(6):
                nc.tensor.matmul(wtile[0:128, 0:128], eyp_s[:, 0:128],
                                 eyp_s[:, 0:128], start=True, stop=True,
                                 skip_group_check=True)

            def emit_fused(gA, gB, lhs, ws, P):
                nk = len(lhs)
                for k in range(nk):
                    st, sp = k == 0, k == nk - 1
                    for gt, wc in ((gA, 0), (gA, 512), (gB, 1024), (gB, 1536)):
                        oc = wc % 1024
                        nc.tensor.matmul(gt[0:P, oc:oc + 512], lhs[k],
                                         ws[k][:, wc:wc + 512],
                                         start=st, stop=sp,
                                         skip_group_check=True)

            def emit_half(g, lhs, ws, dsel, P):
                nk = len(lhs)
                for k in range(nk):
                    st, sp = k == 0, k == nk - 1
                    for j, wc in enumerate((0, 512, 1024, 1536)):
                        w0 = wc + dsel * 256
                        nc.tensor.matmul(g[0:P, j * 256:(j + 1) * 256], lhs[k],
                                         ws[k][:, w0:w0 + 256],
                                         start=st, stop=sp,
                                         skip_group_check=True)

            def unit_fused(L, pk, row_off):
                P = 128
                c0 = pk * 128
                gA = g_pool.tile([128, 1024], F32, tag="g")
                gB = g_pool.tile([128, 1024], F32, tag="g")
                if L == 13:
                    lhs = [oh3_s[:, c0:c0 + P]]
                    ws = [w13_s]
                    cin_ap = cin13_s[0:P, pk * 512:(pk + 1) * 512]
                else:
                    st = stor[L]
                    lhs = [st.sA0[:, c0:c0 + P], st.sA1[:, c0:c0 + P],
                           st.sB0[:, c0:c0 + P], st.sB1[:, c0:c0 + P],
                           ohs_s[:, OHS_OFF[L] + c0:OHS_OFF[L] + c0 + P]]
                    ws = wk + [woh_s]
                    cin_ap = st.cin[0:P, pk * 512:(pk + 1) * 512]
                emit_fused(gA, gB, lhs, ws, P)

                sg = sig_pool.tile([128, 2048], F16, tag="sg")
                nc.scalar.activation(sg[0:P, 0:1024], gA[0:P], AF.Sigmoid)
                nc.scalar.activation(sg[0:P, 1024:2048], gB[0:P], AF.Sigmoid)
                i_ = sg[0:P, 0:512]
                f_ = sg[0:P, 512:1024]
                o_ = sg[0:P, 1024:1536]
                g_ = sg[0:P, 1536:2048]
                q = cell_pool.tile([128, 512], F16, tag="q")
                nc.gpsimd.tensor_mul(q[0:P], f_, cin_ap)
                p = cell_pool.tile([128, 512], F16, tag="p")
                nc.vector.tensor_mul(p[0:P], i_, g_)
                pr = cell_pool.tile([128, 512], F16, tag="pr")
                nc.vector.scalar_tensor_tensor(pr[0:P], p[0:P], 2.0, i_,
                                               ALU.mult, ALU.subtract)
                cn = cell_pool.tile([128, 512], F16, tag="cn", bufs=3)
                nc.vector.tensor_add(cn[0:P], q[0:P], pr[0:P])
                tc_ = cell_pool.tile([128, 512], F16, tag="tc")
                nc.scalar.activation(tc_[0:P], cn[0:P], AF.Tanh)
                hn = cell_pool.tile([128, 512], F16, tag="hn", bufs=3)
                nc.vector.tensor_mul(hn[0:P], o_, tc_[0:P])
                nc.sync.dma_start(out_d[row_off + c0:row_off + c0 + P, :],
                                  hn[0:P])
                return (hn, cn, P)

            def unit_half(lhs, cin_half, P, dsel, out_ap):
                """Critical (dsel=0) or deferred (dsel=1) half of a small
                level; gates [i f o g] (256 each) in one 1024-col tile."""
                g = g_pool.tile([128, 1024], F32, tag="g")
                emit_half(g, lhs, wk + [woh_s], dsel, P)
                sg = sig_pool.tile([128, 1024], F16, tag="sgh")
                nc.scalar.activation(sg[0:P], g[0:P], AF.Sigmoid)
                i_ = sg[0:P, 0:256]
                f_ = sg[0:P, 256:512]
                o_ = sg[0:P, 512:768]
                gg = sg[0:P, 768:1024]
                p = cell_pool.tile([128, 256], F16, tag="ph")
                nc.vector.tensor_mul(p[0:P], i_, gg)
                pr = cell_pool.tile([128, 256], F16, tag="prh")
                nc.vector.scalar_tensor_tensor(pr[0:P], p[0:P], 2.0, i_,
                                               ALU.mult, ALU.subtract)
                q = cell_pool.tile([128, 256], F16, tag="qh")
                nc.vector.tensor_mul(q[0:P], f_, cin_half)
                cn = cell_pool.tile([128, 256], F16, tag="cnh", bufs=3)
                nc.vector.tensor_add(cn[0:P], q[0:P], pr[0:P])
                tc_ = cell_pool.tile([128, 256], F16, tag="tch")
                nc.scalar.activation(tc_[0:P], cn[0:P], AF.Tanh)
                hn = cell_pool.tile([128, 256], F16, tag="hnh", bufs=3)
                nc.vector.tensor_mul(hn[0:P], o_, tc_[0:P])
                nc.sync.dma_start(out_ap, hn[0:P])
                return (hn, cn, P)

            def feed(parent, u, ci, sm_off=None):
                """Write u's crit states into parent stationary storage."""
                hn, cn, P = u
                half = P // 2
                base = ci * 64
                off = EYP_OFF[P]
                pid = eyp_s[0:P, off:off + P]
                t0 = tr_pool.tile([128, 256], F16, tag="t")
                nc.tensor.transpose(t0[0:128, 0:P], hn[0:P, 0:128], pid)
                t1 = tr_pool.tile([128, 256], F16, tag="t")
                nc.tensor.transpose(t1[0:128, 0:P], hn[0:P, 128:256], pid)
                nc.vector.tensor_copy(parent.sA0[:, base:base + half],
                                      t0[:, 0:half])
                nc.vector.tensor_copy(parent.sB0[:, base:base + half],
                                      t0[:, half:P])
                nc.vector.tensor_copy(parent.sA1[:, base:base + half],
                                      t1[:, 0:half])
                nc.vector.tensor_copy(parent.sB1[:, base:base + half],
                                      t1[:, half:P])
                dr = base % 128
                cb = (ci // 2) * 512
                nc.sync.dma_start(parent.cin[dr:dr + half, cb:cb + 256],
                                  cn[0:P:2, 0:256])
                nc.sync.dma_start(parent.cin[dr:dr + half, cb + 256:cb + 512],
                                  cn[1:P:2, 0:256])
                if sm_off is not None:
                    nc.scalar.dma_start(cin_sm[sm_off:sm_off + half, 0:256],
                                        cn[0:P:2, 0:256])
                    nc.scalar.dma_start(cin_sm[sm_off:sm_off + half, 256:512],
                                        cn[1:P:2, 0:256])

            def small_lhs(L):
                st = stor[L]
                M = 1 << (L - 3)
                return [st.sA0[:, 0:M], st.sA1[:, 0:M], st.sB0[:, 0:M],
                        st.sB1[:, 0:M],
                        ohs_s[:, OHS_OFF[L]:OHS_OFF[L] + M]]

            # ---- emission schedule (feeds lag one unit) ----
            u13 = [None] * 8
            u13[0] = unit_fused(13, 0, 0)
            u13[1] = unit_fused(13, 1, 0)
            for c in range(2, 8):
                feed(stor[12], u13[c - 2], c - 2)
                u13[c] = unit_fused(13, c, 0)
            feed(stor[12], u13[6], 6)
            u12 = [None] * 4
            u12[0] = unit_fused(12, 0, 1024)
            feed(stor[12], u13[7], 7)
            u12[1] = unit_fused(12, 1, 1024)
            feed(stor[11], u12[0], 0)
            u12[2] = unit_fused(12, 2, 1024)
            feed(stor[11], u12[1], 1)
            u12[3] = unit_fused(12, 3, 1024)
            feed(stor[11], u12[2], 2)
            u11_0 = unit_fused(11, 0, 1536)
            feed(stor[11], u12[3], 3)
            u11_1 = unit_fused(11, 1, 1536)
            feed(stor[10], u11_0, 0)
            feed(stor[10], u11_1, 1)

            # small levels: crit halves, feeding down; defers batched after
            u10 = unit_half(small_lhs(10), stor[10].cin[0:128, 0:256], 128,
                            0, out_d[1792:1920, 0:256])
            feed(stor[9], u10, 0, sm_off=SMALL_OFF[9])
            u9 = unit_half(small_lhs(9), stor[9].cin[0:64, 0:256], 64,
                           0, out_d[1920:1984, 0:256])
            feed(stor[8], u9, 0, sm_off=SMALL_OFF[8])
            # deferred L10 half interleaves here: keeps the PE warm through
            # the small-level latency chain and off the kernel tail
            unit_half(small_lhs(10), stor[10].cin[0:128, 256:512], 128,
                      1, out_d[1792:1920, 256:512])
            u8 = unit_half(small_lhs(8), stor[8].cin[0:32, 0:256], 32,
                           0, out_d[1984:2016, 0:256])
            feed(stor[7], u8, 0, sm_off=SMALL_OFF[7])
            u7 = unit_half(small_lhs(7), stor[7].cin[0:16, 0:256], 16,
                           0, out_d[2016:2032, 0:256])
            feed(stor[6], u7, 0, sm_off=SMALL_OFF[6])
            # deferred L9..L6 halves (one 120-row chunk)
            lhs2 = [sA0s[:, 0:SMALL_W], sA1s[:, 0:SMALL_W],
                    sB0s[:, 0:SMALL_W], sB1s[:, 0:SMALL_W],
                    ohs_s[:, 896:896 + SMALL_W]]
            unit_half(lhs2, cin_sm[0:SMALL_W, 256:512], SMALL_W,
                      1, out_d[1920:2040, 256:512])
            u6 = unit_half(small_lhs(6), stor[6].cin[0:8, 0:256], 8,
                           0, out_d[2032:2040, 0:256])
            # level-6 c_crit rows for the host's top-of-tree chain
            nc.sync.dma_start(out_d[2040:2048, 0:256], u6[1][0:8, 0:256])

    nc.compile()
    return [k for k in din]


def _get_built():
    global _BUILT
    if _BUILT is None:
        nc = bacc.Bacc("TRN2", target_bir_lowering=False, debug=False,
                       num_devices=N_CORES)
        names = _build_program(nc)
        _BUILT = (nc, names)
    return _BUILT


def _make_eyp():
    eyp = np.zeros((128, EYP_W), np.float16)
    for P, off in EYP_OFF.items():
        half = P // 2
        for bcol in range(P):
            a = 2 * bcol if bcol < half else 2 * (bcol - half) + 1
            eyp[a, off + bcol] = 1.0
    return eyp


def kernel(types, a_idx, b_idx, emb, W_ih, W_hh, b_ih, b_hh):
    types = np.asarray(types, np.int32)
    emb = np.asarray(emb, np.float32)
    W_ih = np.asarray(W_ih, np.float32)
    W_hh = np.asarray(W_hh, np.float32)
    b = np.asarray(b_ih, np.float32) + np.asarray(b_hh, np.float32)

    # ---- host weight reparameterization (O(V), no O(N) arithmetic) ----
    XT = (W_ih @ emb.T + b[:, None]).astype(np.float32)          # [2048, 32]
    c_leaf = _sigmoid(XT[0:512]) * np.tanh(XT[1024:1536])        # [512, 32]
    h_leaf = _sigmoid(XT[1536:2048]) * np.tanh(c_leaf)           # [512, 32]
    M_A = W_hh[:, 0:256] @ h_leaf[0:256]                         # [2048, 32]
    M_B = W_hh[:, 256:512] @ h_leaf[0:256]

    def dev_layout(mat_t):
        """[K, 2048] original gate cols -> [i|f|o|g], g cols doubled, f16."""
        m = np.ascontiguousarray(mat_t[:, PERM3], np.float32)
        m[:, 1536:2048] *= 2.0
        return m.astype(np.float16)

    w13 = dev_layout(np.vstack([M_A.T, M_B.T, XT.T]))            # [96, 2048]
    W_augT = dev_layout(np.vstack([W_hh.T, XT.T]))               # [544, 2048]
    wkh = [np.ascontiguousarray(W_augT[i * 128:(i + 1) * 128])
           for i in range(4)]
    woh = np.ascontiguousarray(W_augT[512:544])
    cl256 = np.ascontiguousarray(c_leaf[0:256].T, np.float16)    # [32, 256]
    eyp = _make_eyp()

    in_maps = []
    for j in range(N_CORES):
        base13 = (1 << 13) - 1 + j * 1024
        n = np.arange(base13, base13 + 1024)
        oh3 = np.zeros((96, 1024), np.float16)
        m = np.arange(1024)
        oh3[types[2 * n + 1], m] = 1.0
        oh3[32 + types[2 * n + 2], m] = 1.0
        oh3[64 + types[n], m] = 1.0
        cin13 = np.concatenate(
            [cl256[types[2 * n + 1]], cl256[types[2 * n + 2]]],
            axis=1).astype(np.float16)
        ohs = np.zeros((32, OHS_W), np.float16)
        for L in range(12, 5, -1):
            mm = 1 << (L - 3)
            basel = (1 << L) - 1 + j * mm
            off = OHS_OFF[L]
            ohs[types[basel:basel + mm], off + np.arange(mm)] = 1.0
        in_maps.append({
            "wk0": wkh[0], "wk1": wkh[1], "wk2": wkh[2], "wk3": wkh[3],
            "woh": woh, "w13": w13, "cin13": cin13,
            "oh3": oh3, "ohs": ohs, "eyp": eyp,
        })

    nc, _ = _get_built()
    res = run_bass_kernel_spmd(nc, in_maps, core_ids=list(range(N_CORES)))
    global LAST_RESULTS
    LAST_RESULTS = res

    out = np.empty((N, H2), np.float32)
    for j in range(N_CORES):
        r = res.results[j]["out"].astype(np.float32)
        for (L, mm, off) in DEV_PLAN:
            basel = (1 << L) - 1 + j * mm
            out[basel:basel + mm] = r[off:off + mm]
    out[LEAF0:] = h_leaf.T[types[LEAF0:]]

    # top of tree (levels 5..0, 63 nodes) on host, mirroring the reference
    Hs = np.zeros((127, H2), np.float32)
    Cc = np.zeros((127, H), np.float32)  # c_crit only
    for j in range(N_CORES):
        r = res.results[j]["out"].astype(np.float32)
        Hs[63 + 8 * j:63 + 8 * j + 8] = r[2032:2040]
        Cc[63 + 8 * j:63 + 8 * j + 8] = r[2040:2048, 0:256]
    for L in range(5, -1, -1):
        n = np.arange((1 << L) - 1, (1 << (L + 1)) - 1)
        a, bb = 2 * n + 1, 2 * n + 2
        h_in = np.concatenate([Hs[a, :H], Hs[bb, :H]], axis=1)
        c_in = np.concatenate([Cc[a], Cc[bb]], axis=1)
        gates = XT[:, types[n]].T + h_in @ W_hh.T
        ig, fg, gg, og = np.split(gates, 4, axis=1)
        c_new = _sigmoid(fg) * c_in + _sigmoid(ig) * np.tanh(gg)
        h_new = _sigmoid(og) * np.tanh(c_new)
        Hs[n] = h_new
        Cc[n] = c_new[:, 0:256]
        out[n] = h_new
    return out


# revision 17
# speedup vs baseline: 2.0211x; 1.3811x over previous
"""Binary tree-LSTM (BinaryTokenTreeModel) Trainium2 kernel, v5.

Problem: complete binary tree, depth 15 (N=32767 nodes), tree-LSTM with
state size 2H=512, gates 4*2H=2048, vocab 32.  Reference processes nodes
leaves-first; node i's input state is the concat of the first H=256 dims
of its two children's states.

Strategy (8 NeuronCores):
  * Data-parallel over 8 subtrees rooted at the 8 level-3 nodes (7..14).
    Each core runs a level-synchronous scan over global levels 13..10 of
    its subtree (1920 nodes per core, 93.75% of the tree with the leaf
    level).  Host: leaf level (32-entry type table, zero arithmetic) and
    the inherently-serial 1023-node top (levels 9..0, exact fp32).
  * VOCAB=32 => x_proj folded into the level matmul as a one-hot
    contraction block (K = 256+256+32 = 544).  Level 13 contracts K=96
    one-hots against a reparameterized table (children are leaves).
  * sigma-everywhere gates: tanh(x) = 2*sigmoid(2x) - 1 with the 2x folded
    into the g-gate weight columns, minimizing ACT instructions (352-cycle
    fixed cost each); real Tanh only for c_new.
  * Gate column layout [i | f | o | g] (each 512 = crit 256 | defer 256) so
    every cell op is a flat contiguous f16 slice.
  * Level 10 computes the critical half first (feeding the host boundary),
    deferred half as a trailing chunk interleaved into the PE idle.
  * PSUM: gates pool 3 x [128,1024]x2banks, transpose scratch pool
    2 x 1 bank -- feeds never block the matmul ring.
  * Feed transposes use permuted identities so A/B-child columns come out
    blocked; all feed copies are contiguous.  f16 everywhere off-PSUM.

Self-contained: hardcodes all shapes; only needs numpy + the concourse
(bass) toolchain that ships with the environment.
"""

import sys

for _p in ("/opt/trn_rl_repo", "/root/.axon_site/_ro/trn_rl_repo"):
    if _p not in sys.path:
        sys.path.append(_p)

import numpy as np

import concourse.bacc as bacc
import concourse.mybir as mybir
import concourse.tile as tile
from concourse.bass_utils import run_bass_kernel_spmd

F32 = mybir.dt.float32
F16 = mybir.dt.float16
AF = mybir.ActivationFunctionType
ALU = mybir.AluOpType

N_CORES = 8
N = 32767
H = 256
H2 = 512
G = 2048  # 4 * H2
V = 32
LEAF0 = (1 << 14) - 1  # 16383: first leaf node id

# Gate column order [i | f | o | g]; orig torch row order is i f g o.
PERM3 = np.concatenate([
    np.arange(0, 512), np.arange(512, 1024),
    np.arange(1536, 2048), np.arange(1024, 1536),
])

DEV_PLAN = [(13, 1024, 0), (12, 512, 1024), (11, 256, 1536), (10, 128, 1792)]
OUT_ROWS = 2048  # 1920 h rows + 128 rows of level-10 c_crit (cols 0:256)
OHS_OFF = {12: 0, 11: 512, 10: 768}
OHS_W = 896
EYP_OFF = {128: 0}
EYP_W = 128

_BUILT = None  # cached (nc, input_names)
LAST_RESULTS = None  # BassKernelResults of the most recent run (for profiling)


def _sigmoid(x):
    return 1.0 / (1.0 + np.exp(-x))


class _Stor:
    def __init__(self, sA0, sA1, sB0, sB1, cin):
        self.sA0, self.sA1, self.sB0, self.sB1, self.cin = sA0, sA1, sB0, sB1, cin


def _build_program(nc):
    din = {}
    for name, shape in [
        ("wk0", [128, G]), ("wk1", [128, G]), ("wk2", [128, G]), ("wk3", [128, G]),
        ("woh", [32, G]), ("w13", [96, G]),
        ("oh3", [96, 1024]), ("ohs", [32, OHS_W]),
        ("eyp", [128, EYP_W]), ("cin13", [1024, 512]),
    ]:
        din[name] = nc.dram_tensor(name, shape, F16, kind="ExternalInput").ap()
    out_d = nc.dram_tensor("out", [OUT_ROWS, 512], F16, kind="ExternalOutput").ap()

    sbh = lambda n, sh: nc.alloc_sbuf_tensor(n, sh, F16).ap()
    wk = [sbh(f"wk{i}_s", [128, G]) for i in range(4)]
    woh_s = sbh("woh_s", [32, G])
    w13_s = sbh("w13_s", [96, G])
    oh3_s = sbh("oh3_s", [96, 1024])
    ohs_s = sbh("ohs_s", [32, OHS_W])
    eyp_s = sbh("eyp_s", [128, EYP_W])
    cin13_s = sbh("cin13_s", [128, 8 * 512])

    stor = {}
    for L, M in [(12, 512), (11, 256), (10, 128)]:
        mk = lambda nm: sbh(f"{nm}_{L}", [128, M])
        stor[L] = _Stor(mk("sA0"), mk("sA1"), mk("sB0"), mk("sB1"),
                        sbh(f"cin_{L}", [128, (M // 128) * 512]))

    with tile.TileContext(nc) as tc:
        import contextlib

        with contextlib.ExitStack() as ctx:
            g_pool = ctx.enter_context(
                tc.tile_pool(name="g", bufs=3, space="PSUM"))
            tr_pool = ctx.enter_context(
                tc.tile_pool(name="tr", bufs=2, space="PSUM"))
            sig_pool = ctx.enter_context(tc.tile_pool(name="sig", bufs=4))
            cell_pool = ctx.enter_context(tc.tile_pool(name="cell", bufs=3))

            # input loads spread over three DGE queues; L13's operands first
            nc.gpsimd.dma_start(eyp_s, din["eyp"])
            nc.sync.dma_start(w13_s[0:48], din["w13"][0:48])
            nc.scalar.dma_start(w13_s[48:96], din["w13"][48:96])
            nc.sync.dma_start(oh3_s[0:48], din["oh3"][0:48])
            nc.scalar.dma_start(oh3_s[48:96], din["oh3"][48:96])
            for k in range(8):
                nc.gpsimd.dma_start(cin13_s[:, k * 512:(k + 1) * 512],
                                    din["cin13"][k * 128:(k + 1) * 128, :])
            nc.sync.dma_start(wk[0], din["wk0"])
            nc.scalar.dma_start(wk[1], din["wk1"])
            nc.sync.dma_start(wk[2], din["wk2"])
            nc.scalar.dma_start(wk[3], din["wk3"])
            nc.gpsimd.dma_start(woh_s, din["woh"])
            nc.gpsimd.dma_start(ohs_s, din["ohs"])

            # junk matmuls: occupy the HAM activity window while input DMAs
            # land so the PE unthrottles to 2.4 GHz before the real work
            wtile = g_pool.tile([128, 1024], F32, tag="g")
            for _ in range(16):
                nc.tensor.matmul(wtile[0:128, 0:128], eyp_s[:, 0:128],
                                 eyp_s[:, 0:128], start=True, stop=True,
                                 skip_group_check=True)

            def emit_fused(gA, gB, lhs, ws, P):
                nk = len(lhs)
                for k in range(nk):
                    st, sp = k == 0, k == nk - 1
                    for gt, wc in ((gA, 0), (gA, 512), (gB, 1024), (gB, 1536)):
                        oc = wc % 1024
                        nc.tensor.matmul(gt[0:P, oc:oc + 512], lhs[k],
                                         ws[k][:, wc:wc + 512],
                                         start=st, stop=sp,
                                         skip_group_check=True)

            def emit_half(g, lhs, ws, dsel, P):
                nk = len(lhs)
                for k in range(nk):
                    st, sp = k == 0, k == nk - 1
                    for j, wc in enumerate((0, 512, 1024, 1536)):
                        w0 = wc + dsel * 256
                        nc.tensor.matmul(g[0:P, j * 256:(j + 1) * 256], lhs[k],
                                         ws[k][:, w0:w0 + 256],
                                         start=st, stop=sp,
                                         skip_group_check=True)

            def unit_fused(L, pk, row_off):
                P = 128
                c0 = pk * 128
                gA = g_pool.tile([128, 1024], F32, tag="g")
                gB = g_pool.tile([128, 1024], F32, tag="g")
                if L == 13:
                    lhs = [oh3_s[:, c0:c0 + P]]
                    ws = [w13_s]
                    cin_ap = cin13_s[0:P, pk * 512:(pk + 1) * 512]
                else:
                    st = stor[L]
                    lhs = [st.sA0[:, c0:c0 + P], st.sA1[:, c0:c0 + P],
                           st.sB0[:, c0:c0 + P], st.sB1[:, c0:c0 + P],
                           ohs_s[:, OHS_OFF[L] + c0:OHS_OFF[L] + c0 + P]]
                    ws = wk + [woh_s]
                    cin_ap = st.cin[0:P, pk * 512:(pk + 1) * 512]
                emit_fused(gA, gB, lhs, ws, P)

                sg = sig_pool.tile([128, 2048], F16, tag="sg")
                nc.scalar.activation(sg[0:P, 0:1024], gA[0:P], AF.Sigmoid)
                nc.scalar.activation(sg[0:P, 1024:2048], gB[0:P], AF.Sigmoid)
                i_ = sg[0:P, 0:512]
                f_ = sg[0:P, 512:1024]
                o_ = sg[0:P, 1024:1536]
                g_ = sg[0:P, 1536:2048]
                q = cell_pool.tile([128, 512], F16, tag="q")
                nc.gpsimd.tensor_mul(q[0:P], f_, cin_ap)
                p = cell_pool.tile([128, 512], F16, tag="p")
                nc.vector.tensor_mul(p[0:P], i_, g_)
                pr = cell_pool.tile([128, 512], F16, tag="pr")
                nc.vector.scalar_tensor_tensor(pr[0:P], p[0:P], 2.0, i_,
                                               ALU.mult, ALU.subtract)
                cn = cell_pool.tile([128, 512], F16, tag="cn", bufs=3)
                nc.vector.tensor_add(cn[0:P], q[0:P], pr[0:P])
                tc_ = cell_pool.tile([128, 512], F16, tag="tc")
                nc.scalar.activation(tc_[0:P], cn[0:P], AF.Tanh)
                hn = cell_pool.tile([128, 512], F16, tag="hn", bufs=3)
                nc.vector.tensor_mul(hn[0:P], o_, tc_[0:P])
                nc.sync.dma_start(out_d[row_off + c0:row_off + c0 + P, :],
                                  hn[0:P])
                return (hn, cn, P)

            def unit_half(lhs, cin_half, P, dsel, out_ap):
                """Critical (dsel=0) or deferred (dsel=1) half of level 10;
                gates [i f o g] (256 each) in one 1024-col tile."""
                g = g_pool.tile([128, 1024], F32, tag="g")
                emit_half(g, lhs, wk + [woh_s], dsel, P)
                sg = sig_pool.tile([128, 1024], F16, tag="sgh")
                nc.scalar.activation(sg[0:P], g[0:P], AF.Sigmoid)
                i_ = sg[0:P, 0:256]
                f_ = sg[0:P, 256:512]
                o_ = sg[0:P, 512:768]
                gg = sg[0:P, 768:1024]
                p = cell_pool.tile([128, 256], F16, tag="ph")
                nc.vector.tensor_mul(p[0:P], i_, gg)
                pr = cell_pool.tile([128, 256], F16, tag="prh")
                nc.vector.scalar_tensor_tensor(pr[0:P], p[0:P], 2.0, i_,
                                               ALU.mult, ALU.subtract)
                q = cell_pool.tile([128, 256], F16, tag="qh")
                nc.vector.tensor_mul(q[0:P], f_, cin_half)
                cn = cell_pool.tile([128, 256], F16, tag="cnh", bufs=3)
                nc.vector.tensor_add(cn[0:P], q[0:P], pr[0:P])
                tc_ = cell_pool.tile([128, 256], F16, tag="tch")
                nc.scalar.activation(tc_[0:P], cn[0:P], AF.Tanh)
                hn = cell_pool.tile([128, 256], F16, tag="hnh", bufs=3)
                nc.vector.tensor_mul(hn[0:P], o_, tc_[0:P])
                nc.sync.dma_start(out_ap, hn[0:P])
                return (hn, cn, P)

            def feed(parent, u, ci):
                """Write u's crit states into parent stationary storage."""
                hn, cn, P = u
                half = P // 2
                base = ci * 64
                pid = eyp_s[0:P, 0:P]
                t0 = tr_pool.tile([128, 256], F16, tag="t")
                nc.tensor.transpose(t0[0:128, 0:P], hn[0:P, 0:128], pid)
                t1 = tr_pool.tile([128, 256], F16, tag="t")
                nc.tensor.transpose(t1[0:128, 0:P], hn[0:P, 128:256], pid)
                nc.vector.tensor_copy(parent.sA0[:, base:base + half],
                                      t0[:, 0:half])
                nc.vector.tensor_copy(parent.sB0[:, base:base + half],
                                      t0[:, half:P])
                nc.vector.tensor_copy(parent.sA1[:, base:base + half],
                                      t1[:, 0:half])
                nc.vector.tensor_copy(parent.sB1[:, base:base + half],
                                      t1[:, half:P])
                dr = base % 128
                cb = (ci // 2) * 512
                nc.sync.dma_start(parent.cin[dr:dr + half, cb:cb + 256],
                                  cn[0:P:2, 0:256])
                nc.sync.dma_start(parent.cin[dr:dr + half, cb + 256:cb + 512],
                                  cn[1:P:2, 0:256])

            # ---- emission schedule (feeds lag one unit) ----
            u13 = [None] * 8
            u13[0] = unit_fused(13, 0, 0)
            u13[1] = unit_fused(13, 1, 0)
            for c in range(2, 8):
                feed(stor[12], u13[c - 2], c - 2)
                u13[c] = unit_fused(13, c, 0)
            feed(stor[12], u13[6], 6)
            u12 = [None] * 4
            u12[0] = unit_fused(12, 0, 1024)
            feed(stor[12], u13[7], 7)
            u12[1] = unit_fused(12, 1, 1024)
            feed(stor[11], u12[0], 0)
            u12[2] = unit_fused(12, 2, 1024)
            feed(stor[11], u12[1], 1)
            u12[3] = unit_fused(12, 3, 1024)
            feed(stor[11], u12[2], 2)
            u11_0 = unit_fused(11, 0, 1536)
            feed(stor[11], u12[3], 3)
            u11_1 = unit_fused(11, 1, 1536)
            feed(stor[10], u11_0, 0)
            feed(stor[10], u11_1, 1)

            # level 10: critical half (host boundary) then deferred half
            lhs10 = [stor[10].sA0, stor[10].sA1, stor[10].sB0, stor[10].sB1,
                     ohs_s[:, OHS_OFF[10]:OHS_OFF[10] + 128]]
            u10 = unit_half(lhs10, stor[10].cin[0:128, 0:256], 128,
                            0, out_d[1792:1920, 0:256])
            unit_half(lhs10, stor[10].cin[0:128, 256:512], 128,
                      1, out_d[1792:1920, 256:512])
            # level-10 c_crit rows for the host's top-of-tree chain
            nc.sync.dma_start(out_d[1920:2048, 0:256], u10[1][0:128, 0:256])

    nc.compile()
    return [k for k in din]


def _get_built():
    global _BUILT
    if _BUILT is None:
        nc = bacc.Bacc("TRN2", target_bir_lowering=False, debug=False,
                       num_devices=N_CORES)
        names = _build_program(nc)
        _BUILT = (nc, names)
    return _BUILT


def _make_eyp():
    eyp = np.zeros((128, EYP_W), np.float16)
    for P, off in EYP_OFF.items():
        half = P // 2
        for bcol in range(P):
            a = 2 * bcol if bcol < half else 2 * (bcol - half) + 1
            eyp[a, off + bcol] = 1.0
    return eyp


def kernel(types, a_idx, b_idx, emb, W_ih, W_hh, b_ih, b_hh):
    types = np.asarray(types, np.int32)
    emb = np.asarray(emb, np.float32)
    W_ih = np.asarray(W_ih, np.float32)
    W_hh = np.asarray(W_hh, np.float32)
    b = np.asarray(b_ih, np.float32) + np.asarray(b_hh, np.float32)

    # ---- host weight reparameterization (O(V), no O(N) arithmetic) ----
    XT = (W_ih @ emb.T + b[:, None]).astype(np.float32)          # [2048, 32]
    c_leaf = _sigmoid(XT[0:512]) * np.tanh(XT[1024:1536])        # [512, 32]
    h_leaf = _sigmoid(XT[1536:2048]) * np.tanh(c_leaf)           # [512, 32]
    M_A = W_hh[:, 0:256] @ h_leaf[0:256]                         # [2048, 32]
    M_B = W_hh[:, 256:512] @ h_leaf[0:256]

    def dev_layout(mat_t):
        """[K, 2048] original gate cols -> [i|f|o|g], g cols doubled, f16."""
        m = np.ascontiguousarray(mat_t[:, PERM3], np.float32)
        m[:, 1536:2048] *= 2.0
        return m.astype(np.float16)

    w13 = dev_layout(np.vstack([M_A.T, M_B.T, XT.T]))            # [96, 2048]
    W_augT = dev_layout(np.vstack([W_hh.T, XT.T]))               # [544, 2048]
    wkh = [np.ascontiguousarray(W_augT[i * 128:(i + 1) * 128])
           for i in range(4)]
    woh = np.ascontiguousarray(W_augT[512:544])
    cl256 = np.ascontiguousarray(c_leaf[0:256].T, np.float16)    # [32, 256]
    eyp = _make_eyp()

    in_maps = []
    for j in range(N_CORES):
        base13 = (1 << 13) - 1 + j * 1024
        n = np.arange(base13, base13 + 1024)
        oh3 = np.zeros((96, 1024), np.float16)
        m = np.arange(1024)
        oh3[types[2 * n + 1], m] = 1.0
        oh3[32 + types[2 * n + 2], m] = 1.0
        oh3[64 + types[n], m] = 1.0
        cin13 = np.concatenate(
            [cl256[types[2 * n + 1]], cl256[types[2 * n + 2]]],
            axis=1).astype(np.float16)
        ohs = np.zeros((32, OHS_W), np.float16)
        for L in range(12, 9, -1):
            mm = 1 << (L - 3)
            basel = (1 << L) - 1 + j * mm
            off = OHS_OFF[L]
            ohs[types[basel:basel + mm], off + np.arange(mm)] = 1.0
        in_maps.append({
            "wk0": wkh[0], "wk1": wkh[1], "wk2": wkh[2], "wk3": wkh[3],
            "woh": woh, "w13": w13, "cin13": cin13,
            "oh3": oh3, "ohs": ohs, "eyp": eyp,
        })

    nc, _ = _get_built()
    res = run_bass_kernel_spmd(nc, in_maps, core_ids=list(range(N_CORES)))
    global LAST_RESULTS
    LAST_RESULTS = res

    out = np.empty((N, H2), np.float32)
    for j in range(N_CORES):
        r = res.results[j]["out"].astype(np.float32)
        for (L, mm, off) in DEV_PLAN:
            basel = (1 << L) - 1 + j * mm
            out[basel:basel + mm] = r[off:off + mm]
    out[LEAF0:] = h_leaf.T[types[LEAF0:]]

    # top of tree (levels 9..0, 1023 nodes) on host, mirroring the reference
    NB = (1 << 11) - 1  # nodes 0..2046 (level-10 boundary included)
    Hs = np.zeros((NB, H2), np.float32)
    Cc = np.zeros((NB, H), np.float32)  # c_crit only
    for j in range(N_CORES):
        r = res.results[j]["out"].astype(np.float32)
        b10 = (1 << 10) - 1 + 128 * j
        Hs[b10:b10 + 128] = r[1792:1920]
        Cc[b10:b10 + 128] = r[1920:2048, 0:256]
    for L in range(9, -1, -1):
        n = np.arange((1 << L) - 1, (1 << (L + 1)) - 1)
        a, bb = 2 * n + 1, 2 * n + 2
        h_in = np.concatenate([Hs[a, :H], Hs[bb, :H]], axis=1)
        c_in = np.concatenate([Cc[a], Cc[bb]], axis=1)
        gates = XT[:, types[n]].T + h_in @ W_hh.T
        ig, fg, gg, og = np.split(gates, 4, axis=1)
        c_new = _sigmoid(fg) * c_in + _sigmoid(ig) * np.tanh(gg)
        h_new = _sigmoid(og) * np.tanh(c_new)
        Hs[n] = h_new
        Cc[n] = c_new[:, 0:256]
        out[n] = h_new
    return out
